# revision 3
# baseline (speedup 1.0000x reference)
"""Multi-head causal attention with RoPE on 8 Trainium2 NeuronCores.

Sharding: batch x head-group tensor parallel. Core c owns batch c//4 and
heads 4*(c%4)..4*(c%4)+3 (two pairs of 2 heads packed into 128 partitions).
QKV projection is column-sliced per core, attention fully local per head,
and the output projection is row-parallel: each core emits a full-shape
[T, D] fp16 partial for its batch; the host sums 4 partials per batch.

All matmul operands are fp16 (1 cycle/row at any width; fp32 PSUM accum).
On-device layout is fully "transposed": q,k live as [head_dim, T]; scores
are built as S^T[k,q] tiles with both heads of a pair packed side by side
in one [128,1024] PSUM tile so a single exp instruction covers both. The
PV matmul consumes exp(S^T) with v in natural [T, head_dim] layout and a
fused ones-column producing the softmax denominator as pso row 64.

Instruction emission is software-pipelined: QKV/RoPE/V of q-block j+1 and
the output projection of block j-1 are interleaved into attention block
j's matmul stream so the PE queue never drains (keeps the 2.4 GHz ramp).
"""
import sys
from collections import deque

sys.path.insert(0, "/opt/trn_rl_repo")

import numpy as np

B, T, D, H, HD = 2, 2048, 1024, 16, 64
NCORES = 8
QT = 512  # q-tile width (S^T free dim)
KT = 128  # k-tile width (S^T partition dim)
NQT = T // QT  # 4
NKT = T // KT  # 16
DT = 128  # d-chunk (contraction tiles)
NDT = D // DT  # 8

ROPE_ON_POOL = True  # t2-mul + add of RoPE on GpSimd instead of DVE
MASK_ON_POOL = True  # causal trimask muls on GpSimd

_CACHE = {}


def _build():
    import concourse.bass as bass  # noqa: F401
    from concourse import bacc
    import concourse.mybir as mybir
    from concourse.tile import TileContext

    F32 = mybir.dt.float32
    F16 = mybir.dt.float16
    AF = mybir.ActivationFunctionType

    nc = bacc.Bacc("TRN2", target_bir_lowering=False)

    XT = nc.dram_tensor("xt", [D, T], F16, kind="ExternalInput")
    WQK = nc.dram_tensor("wqk", [D, 512], F16, kind="ExternalInput")
    WV = nc.dram_tensor("wv", [D, 256], F16, kind="ExternalInput")
    WOUT = nc.dram_tensor("wout", [256, D], F16, kind="ExternalInput")
    COS2 = nc.dram_tensor("cos2", [128, T], F16, kind="ExternalInput")
    SIN2 = nc.dram_tensor("sin2", [128, T], F16, kind="ExternalInput")
    P2T = nc.dram_tensor("p2t", [128, 128], F16, kind="ExternalInput")
    TRIMASK = nc.dram_tensor("trimask", [128, 128], F16, kind="ExternalInput")
    ONESBC = nc.dram_tensor("onesbc", [1, 64], F16, kind="ExternalInput")
    OUTP = nc.dram_tensor("outp", [T, D], F16, kind="ExternalOutput")

    pool_eng = None  # set inside context

    with TileContext(nc) as tc:
        with (
            tc.tile_pool(name="const", bufs=1) as cst,
            tc.tile_pool(name="xtp", bufs=1) as xtp,
            tc.tile_pool(name="qk", bufs=1) as qkp,
            tc.tile_pool(name="vp", bufs=1) as vp,
            tc.tile_pool(name="ot", bufs=1) as otp,
            tc.tile_pool(name="pt", bufs=4) as ptp,
            tc.tile_pool(name="rt", bufs=2) as rtp,
            tc.tile_pool(name="sm", bufs=3) as smp,
            tc.tile_pool(name="ost", bufs=3) as osp,
            tc.tile_pool(name="psS", bufs=2, space="PSUM") as psS,  # 2x[128,1024] = 4 banks
            tc.tile_pool(name="psO", bufs=2, space="PSUM") as psO,  # 2x[65,512]  = 2 banks
            tc.tile_pool(name="psF", bufs=2, space="PSUM") as psF,  # 2x[128,512] = 2 banks
        ):
            rope_eng = nc.gpsimd if ROPE_ON_POOL else nc.vector
            mask_eng = nc.gpsimd if MASK_ON_POOL else nc.vector

            # ---- persistent tiles ----
            cos2 = cst.tile([128, T], F16, tag="cos2")
            sin2 = cst.tile([128, T], F16, tag="sin2")
            p2t = cst.tile([128, 128], F16, tag="p2t")
            trimask = cst.tile([128, 128], F16, tag="trimask")
            onesbc = cst.tile([1, 64], F16, tag="onesbc")
            wout_c = [cst.tile([128, D], F16, tag=f"wout{p}", name=f"wout{p}") for p in range(2)]
            wqk = [cst.tile([DT, 512], F16, tag=f"wqk{d}", name=f"wqk{d}") for d in range(NDT)]
            wv = [cst.tile([DT, 256], F16, tag=f"wv{d}", name=f"wv{d}") for d in range(NDT)]
            xtt = [
                [xtp.tile([DT, QT], F16, tag=f"xt{d}_{tj}", name=f"xt{d}_{tj}") for tj in range(NQT)]
                for d in range(NDT)
            ]
            # q,k per pair: [128(2 heads x 64), T]; index [pair][0=q,1=k]
            qkt = [
                [qkp.tile([128, T], F16, tag=f"qk{p}{w}", name=f"qk{p}{w}") for w in range(2)]
                for p in range(2)
            ]
            # v natural layout per pair/k-tile: [128 t, 130] = [v_h0|1|v_h1|1]
            vt = [
                [vp.tile([128, 130], F16, tag=f"v{p}_{i}", name=f"v{p}_{i}") for i in range(NKT)]
                for p in range(2)
            ]
            ot = [otp.tile([128, T], F16, tag=f"ot{p}", name=f"ot{p}") for p in range(2)]

            # ---- zero the two S-psum ring slots (exp reads stale "holes"
            # in diagonal groups; they must be bounded values, never junk)
            sg_init = [
                psS.tile([128, 1024], F32, tag="sg", name=f"sginit{k}")
                for k in range(2)
            ]
            for t_ in sg_init:
                nc.vector.memset(t_[:], 0.0)
            # warm up the exp activation table while DMAs run
            warm = smp.tile([1, 4], F16, tag="warm")
            nc.scalar.activation(warm[:], sg_init[0][0:1, 0:4], AF.Exp, scale=0.125)

            # ---- input DMAs (sync queue, front-loaded in consumption order)
            for d in range(NDT):
                nc.sync.dma_start(wqk[d][:], WQK[d * DT : (d + 1) * DT, :])
            for d in range(NDT):
                nc.sync.dma_start(xtt[d][0][:], XT[d * DT : (d + 1) * DT, 0:QT])
            for d in range(NDT):
                nc.sync.dma_start(wv[d][:], WV[d * DT : (d + 1) * DT, :])
            nc.sync.dma_start(p2t[:], P2T[:])
            nc.sync.dma_start(cos2[:], COS2[:])
            nc.sync.dma_start(sin2[:], SIN2[:])
            nc.sync.dma_start(trimask[:], TRIMASK[:])
            nc.sync.dma_start(onesbc[:], ONESBC[:])
            for tj in range(1, NQT):
                for d in range(NDT):
                    nc.sync.dma_start(
                        xtt[d][tj][:], XT[d * DT : (d + 1) * DT, tj * QT : (tj + 1) * QT]
                    )
            for p in range(2):
                nc.sync.dma_start(wout_c[p][:], WOUT[128 * p : 128 * p + 128, :])
            # ones columns of the v tiles (positions 64 and 129) come from a
            # one-time fill; the projection copies overwrite the v parts
            for p in range(2):
                for i in range(NKT):
                    nc.vector.memset(vt[p][i][:, 64:65], 1.0)
                    nc.vector.memset(vt[p][i][:, 129:130], 1.0)

            # ---- emitters ----
            def qkv_qk_group(tj, pg, w):
                # q (w=0) / k (w=1) for head-pair pg, t-slice tj
                colbase = 256 * pg + 128 * w
                ps = psF.tile([128, QT], F32, tag="f")
                for d in range(NDT):
                    nc.tensor.matmul(
                        ps[:],
                        wqk[d][:, colbase : colbase + 128],
                        xtt[d][tj][:],
                        start=(d == 0),
                        stop=(d == NDT - 1),
                    )
                nc.vector.tensor_copy(qkt[pg][w][:, tj * QT : (tj + 1) * QT], ps[:])

            def v_tile(ti):
                tj, tc_ = divmod(ti, 4)
                ps = psF.tile([128, QT], F32, tag="f")
                for d in range(NDT):
                    nc.tensor.matmul(
                        ps[:, 0:256],
                        xtt[d][tj][:, tc_ * 128 : (tc_ + 1) * 128],
                        wv[d][:],
                        start=(d == 0),
                        stop=(d == NDT - 1),
                    )
                for p in range(2):
                    nc.vector.tensor_copy(
                        vt[p][ti][:, 0:64], ps[:, 128 * p : 128 * p + 64]
                    )
                    nc.vector.tensor_copy(
                        vt[p][ti][:, 65:129], ps[:, 128 * p + 64 : 128 * p + 128]
                    )

            def rope(tj, pg, w):
                raw = qkt[pg][w]
                sl = slice(tj * QT, (tj + 1) * QT)
                psr = psF.tile([128, QT], F32, tag="f")
                nc.tensor.matmul(psr[:], p2t[:], raw[:, sl], start=True, stop=True)
                t1 = rtp.tile([128, QT], F16, tag="t1")
                nc.vector.tensor_mul(t1[:], psr[:], sin2[:, sl])
                t2 = rtp.tile([128, QT], F16, tag="t2")
                rope_eng.tensor_mul(t2[:], raw[:, sl], cos2[:, sl])
                rope_eng.tensor_add(raw[:, sl], t1[:], t2[:])

            def oproj_chunk(j, tc_):
                rows = slice(j * QT + tc_ * 128, j * QT + (tc_ + 1) * 128)
                st = osp.tile([128, D], F16, tag="ost")
                for g2 in range(2):
                    ps = psF.tile([128, QT], F32, tag="f")
                    for p in range(2):
                        nc.tensor.matmul(
                            ps[:],
                            ot[p][:, rows],
                            wout_c[p][:, g2 * QT : (g2 + 1) * QT],
                            start=(p == 0),
                            stop=(p == 1),
                        )
                    nc.vector.tensor_copy(st[:, g2 * QT : (g2 + 1) * QT], ps[:])
                nc.sync.dma_start(OUTP[rows, :], st[:])

            fill = deque()

            def pump(n):
                for _ in range(n):
                    if not fill:
                        return
                    fill.popleft()()

            def attn(j):
                jq = j * QT
                nk = 4 * j + 4
                with nc.named_scope(f"attn{j}"):
                    for p in range(2):
                        pso = [
                            psO.tile([65, QT], F32, tag="o", name=f"pso{h}")
                            for h in range(2)
                        ]
                        for i in range(nk):
                            r = i - 4 * j
                            off = 128 * r if r >= 0 else 0
                            sg = psS.tile([128, 1024], F32, tag="sg")
                            for h in range(2):
                                nc.tensor.matmul(
                                    sg[:, 512 * h + off : 512 * h + 512],
                                    qkt[p][1][64 * h : 64 * h + 64, i * KT : (i + 1) * KT],
                                    qkt[p][0][64 * h : 64 * h + 64, jq + off : jq + QT],
                                    start=True,
                                    stop=True,
                                )
                            pt = ptp.tile([128, 1024], F16, tag="pt")
                            nc.scalar.activation(
                                pt[:, off:1024], sg[:, off:1024], AF.Exp, scale=0.125
                            )
                            if r >= 0:
                                for h in range(2):
                                    o2 = 512 * h + off
                                    mask_eng.tensor_mul(
                                        pt[:, o2 : o2 + 128],
                                        pt[:, o2 : o2 + 128],
                                        trimask[:],
                                    )
                            for h in range(2):
                                nc.tensor.matmul(
                                    pso[h][:, off:QT],
                                    vt[p][i][:, 65 * h : 65 * h + 65],
                                    pt[:, 512 * h + off : 512 * h + 512],
                                    start=(i == 0),
                                    stop=(i == nk - 1),
                                )
                            pump(2)
                        for h in range(2):
                            rcr = smp.tile([1, QT], F16, tag="rcr")
                            nc.vector.tensor_copy(rcr[:], pso[h][64:65, :])
                            psb = psF.tile([128, QT], F32, tag="f")
                            nc.tensor.matmul(
                                psb[0:64, :], onesbc[:], rcr[:], start=True, stop=True
                            )
                            rec = smp.tile([64, QT], F32, tag="rec")
                            nc.vector.reciprocal_approx_fast(rec[:], psb[0:64, :])
                            nc.vector.tensor_mul(
                                ot[p][64 * h : 64 * h + 64, jq : jq + QT],
                                pso[h][0:64, :],
                                rec[:],
                            )
                            pump(1)

            # ---- schedule ----
            def emit_block(tj):
                # QKV + RoPE + V for t-slice tj, in dependency order
                for w in range(2):
                    qkv_qk_group(tj, 0, w)
                for w in range(2):
                    rope(tj, 0, w)
                for w in range(2):
                    qkv_qk_group(tj, 1, w)
                for w in range(2):
                    rope(tj, 1, w)
                for tc_ in range(4):
                    v_tile(4 * tj + tc_)

            with nc.named_scope("qkv0"):
                emit_block(0)
            for j in range(NQT):
                if j + 1 < NQT:
                    tjn = j + 1
                    for w in range(2):
                        fill.append(lambda tjn=tjn, w=w: qkv_qk_group(tjn, 0, w))
                    for w in range(2):
                        fill.append(lambda tjn=tjn, w=w: rope(tjn, 0, w))
                    for w in range(2):
                        fill.append(lambda tjn=tjn, w=w: qkv_qk_group(tjn, 1, w))
                    for w in range(2):
                        fill.append(lambda tjn=tjn, w=w: rope(tjn, 1, w))
                    for tc_ in range(4):
                        fill.append(lambda tjn=tjn, tc_=tc_: v_tile(4 * tjn + tc_))
                attn(j)
                # everything the next attention block needs must be emitted
                pump(len(fill))
                for tc_ in range(4):
                    fill.append(lambda j=j, tc_=tc_: oproj_chunk(j, tc_))
            pump(len(fill))

    nc.compile()
    return nc


def _host_consts():
    pos = np.arange(T, dtype=np.float64)
    theta = 1.0 / (10000.0 ** (np.arange(0, HD, 2, dtype=np.float64) / HD))
    ang = pos[:, None] * theta[None, :]  # [T, 32]
    cos = np.tile(np.cos(ang), (1, 2)).T.astype(np.float16)  # [64, T]
    sin = np.tile(np.sin(ang), (1, 2)).T.astype(np.float16)
    cos2 = np.vstack([cos, cos])  # [128, T] two heads stacked
    sin2 = np.vstack([sin, sin])
    # rotate-half as a matmul: rot = P @ q for q in [64, t] column layout
    P = np.zeros((HD, HD), dtype=np.float32)
    for i_ in range(32):
        P[i_, i_ + 32] = -1.0
        P[i_ + 32, i_] = 1.0
    P2 = np.zeros((128, 128), dtype=np.float32)
    P2[0:64, 0:64] = P
    P2[64:128, 64:128] = P
    p2t = np.ascontiguousarray(P2.T).astype(np.float16)
    f, p = np.meshgrid(np.arange(128), np.arange(128))
    trimask = (p <= f).astype(np.float16)  # [k, q] valid iff k <= q
    onesbc = np.ones((1, 64), dtype=np.float16)
    return cos2, sin2, p2t, trimask, onesbc


def kernel(x, w_qkv, w_out, b_out):
    from concourse.bass_utils import run_bass_kernel_spmd

    if "nc" not in _CACHE:
        _CACHE["nc"] = _build()
    nc = _CACHE["nc"]

    x = np.asarray(x, dtype=np.float32)
    w_qkv = np.asarray(w_qkv, dtype=np.float32)
    w_out = np.asarray(w_out, dtype=np.float32)
    b_out = np.asarray(b_out, dtype=np.float32)

    xt_b = [
        np.ascontiguousarray(x[b].T).astype(np.float16) for b in range(B)
    ]  # [D, T] per batch
    cos2, sin2, p2t, trimask, onesbc = _host_consts()

    wq = w_qkv[:, 0:D]
    wk = w_qkv[:, D : 2 * D]
    wv_full = w_qkv[:, 2 * D : 3 * D]

    in_maps = []
    for c in range(NCORES):
        b, g = divmod(c, 4)
        heads = [4 * g + k for k in range(4)]
        cols = []
        for pg in range(2):
            h0, h1 = heads[2 * pg], heads[2 * pg + 1]
            for w in (wq, wk):
                cols.append(w[:, h0 * HD : (h0 + 1) * HD])
                cols.append(w[:, h1 * HD : (h1 + 1) * HD])
        wqk_c = np.concatenate(cols, axis=1).astype(np.float16)  # [D, 512]
        wv_c = np.concatenate(
            [wv_full[:, h * HD : (h + 1) * HD] for h in heads], axis=1
        ).astype(np.float16)  # [D, 256]
        wout_c = np.concatenate(
            [w_out[h * HD : (h + 1) * HD, :] for h in heads], axis=0
        ).astype(np.float16)  # [256, D]
        in_maps.append(
            {
                "xt": xt_b[b],
                "wqk": np.ascontiguousarray(wqk_c),
                "wv": np.ascontiguousarray(wv_c),
                "wout": np.ascontiguousarray(wout_c),
                "cos2": cos2,
                "sin2": sin2,
                "p2t": p2t,
                "trimask": trimask,
                "onesbc": onesbc,
            }
        )

    global _last_in_maps
    _last_in_maps = in_maps
    res = run_bass_kernel_spmd(nc, in_maps, list(range(NCORES)))
    acc = np.zeros((B, T, D), dtype=np.float32)
    for c in range(NCORES):
        acc[c // 4] += res.results[c]["outp"].astype(np.float32)
    acc += b_out
    return acc
